# revision 10
# baseline (speedup 1.0000x reference)
"""Haar DWT (2x2 stride-2 block decomposition) on 8 Trainium2 NeuronCores.

Input x: (32, 3, 512, 512) f32. Outputs (ll, lh, hl, hh): each (32, 3, 256, 256).

Sharding: pure data parallel over the batch dim — 4 images per core, viewed as
12 channel images of 512x512 per core, one channel per iteration.

Hybrid two-path pipeline, alternating per channel, so the DMA descriptor mix
(input 2 KB runs on the PE path vs 8 KB runs on the DVE path) and the compute
load (PE+copies vs DVE+ACT) both stay below the HBM roofline:

PE path (even channels):
  - rows-on-partitions load (2 KB runs); a constant 128x128 +-0.5 butterfly
    weight matrix computes halved vertical row sums (partitions 0..63) and
    diffs (64..127) in one fp32 matmul per 128-row tile;
  - ACT stages even PSUM columns to SBUF (DVE may read only one PSUM operand),
    DVE does 2 stride-2 column combines per tile -> ll|lh and hl|hh stacked
    over partitions.

DVE path (odd channels):
  - fully contiguous load (8 KB runs), partition p holds rows 4p..4p+3;
  - DVE vertical sum/diff via strided row-parity views, halved in place on
    ACT, then 4 stride-2 column combines.

Both paths store one fully contiguous 1 MB tile per channel (8 KB runs) on the
second HWDGE ring; the host decodes each channel's layout with numpy views.
All values are bit-identical to the fp32 two-op reference formulation (the
butterfly weights are powers of two and the zeros contribute exactly 0.0).
"""

import sys

import numpy as np

if "/opt/trn_rl_repo" not in sys.path:
    sys.path.insert(0, "/opt/trn_rl_repo")

from concourse import bacc, bass, mybir
from concourse import tile
from concourse.bass_utils import run_bass_kernel_spmd

N_CORES = 8
B, C, H, W = 32, 3, 512, 512
BPC = B // N_CORES  # images per core
NCH = BPC * C  # channel images per core (12)
P = 128  # SBUF partitions
NT = H // P  # 128-row tiles per channel (4)
HW_OUT = H // 2  # 256

_CACHE = {}


def _butterfly_weights():
    """W[k, m]: m<64 -> 0.5*(row 2m + row 2m+1); m>=64 -> 0.5*(row 2m'+1 - row 2m')."""
    w = np.zeros((P, P), dtype=np.float32)
    for m in range(64):
        w[2 * m, m] = 0.5
        w[2 * m + 1, m] = 0.5
        w[2 * m, 64 + m] = -0.5
        w[2 * m + 1, 64 + m] = 0.5
    return w


def _pe_channel(nc, pool, psum, wt, xa, oa, i, split_load):
    f32 = mybir.dt.float32
    xin = pool.tile([P, NT, W], f32, tag="xpe")
    if split_load:
        for t in range(NT):
            nc.sync.dma_start(out=xin[:, t, :], in_=xa[i, t])
    else:
        # (t, p, w) -> (p, t, w); fully sequential DRAM read, 2 KB runs
        nc.sync.dma_start(out=xin[:], in_=xa[i].transpose([1, 0, 2]))
    outt = pool.tile([P, NT, 2, HW_OUT], f32, tag="outt")
    for t in range(NT):
        pt = psum.tile([P, W], f32)
        nc.tensor.matmul(pt[:], wt[:], xin[:, t, :], start=True, stop=True)
        pv = pt[:].rearrange("p (j two) -> p j two", two=2)
        cp = pool.tile([P, HW_OUT], f32, tag="cp")
        nc.scalar.copy(cp[:], pv[:, :, 0])
        nc.vector.tensor_add(outt[:, t, 0], pv[:, :, 1], cp[:])
        nc.vector.tensor_sub(outt[:, t, 1], pv[:, :, 1], cp[:])
    nc.scalar.dma_start(out=oa[i], in_=outt[:])


def _dve_channel(nc, pool, xa4, oa, i, split_store):
    f32 = mybir.dt.float32
    xin = pool.tile([P, 4, W], f32, tag="xdve")
    # partition p holds rows 4p..4p+3: fully contiguous load, 8 KB runs
    nc.sync.dma_start(out=xin[:], in_=xa4[i])
    xv = xin[:].rearrange("p (k t) w -> p k t w", t=2)
    e = xv[:, :, 0, :]
    o = xv[:, :, 1, :]
    su = pool.tile([P, 2, W], f32, tag="su")
    df = pool.tile([P, 2, W], f32, tag="df")
    nc.vector.tensor_add(su[:], e, o)
    nc.vector.tensor_sub(df[:], o, e)
    nc.scalar.mul(su[:], su[:], 0.5)
    nc.scalar.mul(df[:], df[:], 0.5)
    sv = su[:].rearrange("p k (j t) -> p k j t", t=2)
    dv = df[:].rearrange("p k (j t) -> p k j t", t=2)
    outt = pool.tile([P, 4, 2, HW_OUT], f32, tag="outt")
    nc.vector.tensor_add(outt[:, 0, :, :], sv[:, :, :, 0], sv[:, :, :, 1])
    nc.vector.tensor_add(outt[:, 1, :, :], dv[:, :, :, 0], dv[:, :, :, 1])
    nc.vector.tensor_sub(outt[:, 2, :, :], sv[:, :, :, 1], sv[:, :, :, 0])
    nc.vector.tensor_sub(outt[:, 3, :, :], dv[:, :, :, 1], dv[:, :, :, 0])
    if split_store:
        nc.scalar.dma_start(out=oa[i, :, 0:2], in_=outt[:, 0:2])
        nc.scalar.dma_start(out=oa[i, :, 2:4], in_=outt[:, 2:4])
    else:
        nc.scalar.dma_start(out=oa[i], in_=outt[:])


def _build():
    nc = bacc.Bacc("TRN2", target_bir_lowering=False, debug=False)
    f32 = mybir.dt.float32
    # Two input views of the same host buffer bytes:
    #  PE path:  [NCH, tile, row-in-tile, W]
    #  DVE path: [NCH, P, 4 rows, W]   (same row-major bytes)
    x = nc.dram_tensor("x", [NCH, NT, P, W], f32, kind="ExternalInput")
    w = nc.dram_tensor("w", [P, P], f32, kind="ExternalInput")
    out = nc.dram_tensor("out", [NCH, P, NT, 2, HW_OUT], f32, kind="ExternalOutput")
    xa = x.ap()
    xa4 = xa.rearrange("n t p w -> n (t p) w").rearrange(
        "n (p r) w -> n p r w", r=4
    )
    oa = out.ap()
    with tile.TileContext(nc) as tc:
        with (
            tc.tile_pool(name="p", bufs=4) as pool,
            tc.tile_pool(name="w", bufs=1) as wpool,
            tc.tile_pool(name="ps", bufs=8, space=bass.MemorySpace.PSUM) as psum,
        ):
            wt = wpool.tile([P, P], f32)
            nc.sync.dma_start(out=wt[:], in_=w.ap())
            for i in range(NCH):
                if i % 2 == 0:
                    _pe_channel(nc, pool, psum, wt, xa, oa, i, split_load=(i == 0))
                else:
                    _dve_channel(nc, pool, xa4, oa, i, split_store=(i == NCH - 1))
    nc.compile()
    return nc


def _get_nc():
    if "nc" not in _CACHE:
        _CACHE["nc"] = _build()
    return _CACHE["nc"]


def _decode(full):
    """(cores, NCH, P, NT, 2, j) device layout -> dict of (cores, NCH, 256, 256)."""
    ncore = full.shape[0]
    outs = {nm: np.empty((ncore, NCH, HW_OUT, HW_OUT), np.float32)
            for nm in ("ll", "lh", "hl", "hh")}
    pe_idx = [i for i in range(NCH) if i % 2 == 0]
    dve_idx = [i for i in range(NCH) if i % 2 == 1]
    # PE-path channels: out[ch, p, t, g, j]; p<64,g=0: ll row 64t+p;
    # p>=64,g=0: lh; g=1: hl/hh
    pe = full[:, pe_idx].transpose(0, 1, 3, 2, 4, 5)  # (cores, n, t, p, g, j)
    npe = len(pe_idx)
    outs["ll"][:, pe_idx] = pe[:, :, :, 0:64, 0, :].reshape(ncore, npe, 256, 256)
    outs["lh"][:, pe_idx] = pe[:, :, :, 64:128, 0, :].reshape(ncore, npe, 256, 256)
    outs["hl"][:, pe_idx] = pe[:, :, :, 0:64, 1, :].reshape(ncore, npe, 256, 256)
    outs["hh"][:, pe_idx] = pe[:, :, :, 64:128, 1, :].reshape(ncore, npe, 256, 256)
    # DVE-path channels: out[ch, p, o, k, j]: output o, row 2p+k
    dve = full[:, dve_idx]  # (cores, n, P, o(4), k(2), j)
    for o, nm in enumerate(("ll", "lh", "hl", "hh")):
        outs[nm][:, dve_idx] = dve[:, :, :, o].reshape(ncore, len(dve_idx), 256, 256)
    return outs


def run(x, **spmd_kwargs):
    """Run the DWT on 8 cores; returns (results_tuple, BassKernelResults)."""
    nc = _get_nc()
    xs = np.ascontiguousarray(np.asarray(x, dtype=np.float32)).reshape(
        N_CORES, NCH, NT, P, W
    )
    wmat = _butterfly_weights()
    in_maps = [{"x": xs[i], "w": wmat} for i in range(N_CORES)]
    res = run_bass_kernel_spmd(nc, in_maps, core_ids=list(range(N_CORES)), **spmd_kwargs)
    # per-core out: (NCH, P, NT, 2, HW_OUT)
    full = np.stack([res.results[i]["out"] for i in range(N_CORES)])
    outs = _decode(full)
    result = tuple(
        outs[nm].reshape(B, C, HW_OUT, HW_OUT) for nm in ("ll", "lh", "hl", "hh")
    )
    return result, res


def kernel(x):
    out, _ = run(x)
    return out


# revision 13
# speedup vs baseline: 1.2088x; 1.2088x over previous
"""Haar DWT (2x2 stride-2 block decomposition) on 8 Trainium2 NeuronCores.

Input x: (32, 3, 512, 512) f32. Outputs (ll, lh, hl, hh): each (32, 3, 256, 256).

Sharding: pure data parallel over the batch dim — 4 images per core, viewed as
12 channel images of 512x512 per core, one channel per iteration.

The vertical (row-pair) butterfly runs on the TensorEngine: a constant 128x128
weight matrix W maps 128 image rows to 64 halved row-sums (partitions 0..63)
and 64 halved row-diffs (partitions 64..127) in one matmul per 128-row tile
(4 per channel). The weights are +-0.5 (exact powers of two) and all other
entries are exactly zero, so the result is bit-identical to the fp32 two-op
formulation. The horizontal stride-2 column combine is then just 2 DVE ops per
tile — (even+odd) producing ll|lh stacked over partitions, and (odd-even)
producing hl|hh — reading PSUM, writing a stacked SBUF tile stored with one
fully contiguous 1 MB DMA per channel.

ACT does no elementwise work and issues the store DMAs on the second HWDGE
ring, overlapping the load ring on Sync.
"""

import sys

import numpy as np

if "/opt/trn_rl_repo" not in sys.path:
    sys.path.insert(0, "/opt/trn_rl_repo")

from concourse import bacc, bass, mybir
from concourse import tile
from concourse.bass_utils import run_bass_kernel_spmd

N_CORES = 8
B, C, H, W = 32, 3, 512, 512
BPC = B // N_CORES  # images per core
NCH = BPC * C  # channel images per core (12)
P = 128  # SBUF partitions
NT = H // P  # 128-row tiles per channel (4)
HW_OUT = H // 2  # 256

_CACHE = {}


def _butterfly_weights():
    """W[k, m]: m<64 -> 0.5*(row 2m + row 2m+1); m>=64 -> 0.5*(row 2m'+1 - row 2m')."""
    w = np.zeros((P, P), dtype=np.float32)
    for m in range(64):
        w[2 * m, m] = 0.5
        w[2 * m + 1, m] = 0.5
        w[2 * m, 64 + m] = -0.5
        w[2 * m + 1, 64 + m] = 0.5
    return w


def _build():
    nc = bacc.Bacc("TRN2", target_bir_lowering=False, debug=False)
    f32 = mybir.dt.float32
    # x viewed as [NCH, tile, row-in-tile, W]
    x = nc.dram_tensor("x", [NCH, NT, P, W], f32, kind="ExternalInput")
    w = nc.dram_tensor("w", [P, P], f32, kind="ExternalInput")
    # out[ch, p, t, g, j]: p<64,g=0: ll row 64t+p | p>=64,g=0: lh row 64t+p-64
    #                      p<64,g=1: hl          | p>=64,g=1: hh
    out = nc.dram_tensor("out", [NCH, P, NT, 2, HW_OUT], f32, kind="ExternalOutput")
    xa = x.ap()
    oa = out.ap()
    with tile.TileContext(nc) as tc:
        with (
            tc.tile_pool(name="p", bufs=6) as pool,
            tc.tile_pool(name="w", bufs=1) as wpool,
            tc.tile_pool(name="ps", bufs=8, space=bass.MemorySpace.PSUM) as psum,
        ):
            wt = wpool.tile([P, P], f32)
            nc.sync.dma_start(out=wt[:], in_=w.ap())
            for i in range(NCH):
                xin = pool.tile([P, NT, W], f32)
                if i == 0:
                    # split the first load so matmuls start ~4 us earlier
                    for t in range(NT):
                        nc.sync.dma_start(out=xin[:, t, :], in_=xa[i, t])
                else:
                    # (t, p, w) -> (p, t, w); fully sequential DRAM read
                    nc.sync.dma_start(out=xin[:], in_=xa[i].transpose([1, 0, 2]))
                outt = pool.tile([P, NT, 2, HW_OUT], f32)
                for t in range(NT):
                    pt = psum.tile([P, W], f32)
                    # stream even columns first, then odd: PSUM holds
                    # [su_e|df_e (0:256), su_o|df_o (256:512)] contiguously,
                    # so the copy and both combines below are unit-stride
                    rhs = xin[:, t, :].rearrange("p (j two) -> p two j", two=2)
                    nc.tensor.matmul(pt[:], wt[:], rhs, start=True, stop=True)
                    pv = pt[:].rearrange("p (two j) -> p two j", two=2)
                    # DVE can read at most one PSUM operand per instruction:
                    # ACT (otherwise idle) stages the even columns into SBUF.
                    cp = pool.tile([P, HW_OUT], f32)
                    nc.scalar.copy(cp[:], pv[:, 0, :])
                    nc.vector.tensor_add(outt[:, t, 0], pv[:, 1, :], cp[:])
                    nc.vector.tensor_sub(outt[:, t, 1], pv[:, 1, :], cp[:])
                if i == NCH - 1:
                    # split the last store so the tail drains in halves
                    nc.scalar.dma_start(out=oa[i, :, 0:2], in_=outt[:, 0:2])
                    nc.scalar.dma_start(out=oa[i, :, 2:4], in_=outt[:, 2:4])
                else:
                    nc.scalar.dma_start(out=oa[i], in_=outt[:])
    nc.compile()
    return nc


def _get_nc():
    if "nc" not in _CACHE:
        _CACHE["nc"] = _build()
    return _CACHE["nc"]


def run(x, **spmd_kwargs):
    """Run the DWT on 8 cores; returns (results_tuple, BassKernelResults)."""
    nc = _get_nc()
    xs = np.ascontiguousarray(np.asarray(x, dtype=np.float32)).reshape(
        N_CORES, NCH, NT, P, W
    )
    wmat = _butterfly_weights()
    in_maps = [{"x": xs[i], "w": wmat} for i in range(N_CORES)]
    res = run_bass_kernel_spmd(nc, in_maps, core_ids=list(range(N_CORES)), **spmd_kwargs)
    # per-core out: (NCH, P, NT, 2, HW_OUT)
    full = np.stack([res.results[i]["out"] for i in range(N_CORES)])
    # -> (cores, NCH, NT, P, 2, j): out image row r = 64*t + (p mod 64)
    full = full.transpose(0, 1, 3, 2, 4, 5)
    def expand(sl):  # (cores, NCH, NT, 64, j) -> (B, C, 256, 256)
        return np.ascontiguousarray(sl).reshape(B, C, HW_OUT, HW_OUT)
    ll = expand(full[:, :, :, 0:64, 0, :])
    lh = expand(full[:, :, :, 64:128, 0, :])
    hl = expand(full[:, :, :, 0:64, 1, :])
    hh = expand(full[:, :, :, 64:128, 1, :])
    return (ll, lh, hl, hh), res


def kernel(x):
    out, _ = run(x)
    return out
